# revision 3
# baseline (speedup 1.0000x reference)
"""Trainium2 Bass kernel for CombinedKSpaceRowwiseMSELoss (v5).

loss = mean((pred-target)^2 over central cols) + mean(|pred-target| over
periphery cols), means over both channels jointly.

Pure data parallel over batch: 4 batches (5120 rows of 640 f32) per core,
sharded B across the 8 cores; host sums the per-core partials.

Perf notes (single-pass latency = HBM stream time + ramp + tail; the
26.2 MB/core of reads is HBM-bound at ~358 GB/s/NC -> ~73 us floor):
- Blocks use a DESCENDING rows-per-partition schedule (10,10,10,5,4,1):
  big early DMAs amortize descriptor posting; late blocks are small so the
  end-of-stream serial chain is short.
- The final 1-row block is additionally COLUMN-SPLIT: [0:400) (left
  periphery + central) lands second-to-last, [400:640) (right periphery)
  lands last. All central sums and the cacc flush therefore complete while
  the last DMA is still draining; the post-stream chain is just a 240-elem
  sub + one 240-elem abs-reduce + the pacc flush.
- pred DMAs ride the SP HWDGE ring, target DMAs the ACT HWDGE ring. The ACT
  engine also runs the central Square+accum, and both queues are in-order —
  so each block's Square is emitted ONE BLOCK LATE (after the next block's
  target dma_start): descriptor posting never waits on compute.
- io pool is triple-buffered (posting runs ~2 blocks ahead); diff/sq
  scratch are single-buffered (reuse hazards are same-engine serial).
- The two accumulator flushes go out on SEPARATE rings so their HBM-write
  receipts overlap.
"""

import sys

for _p in ("/opt/trn_rl_repo",):
    if _p not in sys.path:
        sys.path.insert(0, _p)

import numpy as np
from contextlib import ExitStack

import concourse.bass as bass
import concourse.tile as tile
from concourse import bacc, mybir
from concourse.bass_utils import run_bass_kernel_spmd

N_CORES = 8
B, C, H, W = 32, 2, 640, 640
B_SHARD = B // N_CORES          # 4 batch elements per core
ROWS = B_SHARD * C * H          # 5120 rows per core
P = 128                         # SBUF partitions
CW = int(W * 0.25)              # 160 central cols
CS = (W - CW) // 2              # 240
CE = CS + CW                    # 400
PW = W - CW                     # 480 periphery cols per row

# (rows/partition, col_start, col_end); row base advances only on the last
# column piece of a row-block. Rows sum to ROWS/P = 40.
BLOCKS = (
    (10, 0, W),
    (10, 0, W),
    (10, 0, W),
    (5, 0, W),
    (4, 0, W),
    (1, 0, CE),      # left periphery + central of the last row-block
    (1, CE, W),      # right periphery of the last row-block (tail)
)

F32 = mybir.dt.float32


def _block_pieces(blocks):
    """Assign accumulator columns: returns per-block (central_col or None,
    [periphery cols]) plus the cacc/pacc widths."""
    cacc_w = 0
    pacc_w = 0
    per_block = []
    for r, cs, ce in blocks:
        central = None
        periph = []
        if cs < CS and min(ce, CS) > cs:
            periph.append(pacc_w)
            pacc_w += 1
        if max(cs, CS) < min(ce, CE):
            central = cacc_w
            cacc_w += 1
        if ce > CE and max(cs, CE) < ce:
            periph.append(pacc_w)
            pacc_w += 1
        per_block.append((central, periph))
    return per_block, cacc_w, pacc_w


def build_program(
    loop_n: int = 1,
    blocks: tuple = BLOCKS,
    io_bufs: int = 3,
    work_bufs: int = 1,
    lag: int = 1,
) -> bass.Bass:
    row_total = sum(r for r, cs, ce in blocks if ce == W)
    assert row_total * P == ROWS
    rmax = max(r for r, _, _ in blocks)
    pieces, cacc_w, pacc_w = _block_pieces(blocks)
    NB = len(blocks)

    nc = bacc.Bacc("TRN2", target_bir_lowering=False, debug=False)

    pred = nc.dram_tensor("pred", [ROWS, W], F32, kind="ExternalInput")
    tgt = nc.dram_tensor("target", [ROWS, W], F32, kind="ExternalInput")
    cacc_out = nc.dram_tensor("cacc", [P, cacc_w], F32, kind="ExternalOutput")
    pacc_out = nc.dram_tensor("pacc", [P, pacc_w], F32, kind="ExternalOutput")

    with tile.TileContext(nc) as tc:
        with ExitStack() as ctx:
            io_pool = ctx.enter_context(tc.tile_pool(name="io", bufs=io_bufs))
            work_pool = ctx.enter_context(tc.tile_pool(name="work", bufs=work_bufs))
            acc_pool = ctx.enter_context(tc.tile_pool(name="acc", bufs=1))

            cacc = acc_pool.tile([P, cacc_w], F32)
            pacc = acc_pool.tile([P, pacc_w], F32)

            def emit_central(i, diffs):
                r, cs, ce = blocks[i]
                central_col = pieces[i][0]
                if central_col is None:
                    return
                bw = ce - cs
                d3 = diffs[i][:, : r * bw].rearrange("p (r w) -> p r w", w=bw)
                lo, hi = CS - cs, CE - cs  # central cols within this block
                sq = work_pool.tile([P, rmax * CW], F32, tag="sq")
                nc.scalar.activation(
                    sq[:, : r * CW].rearrange("p (r w) -> p r w", w=CW),
                    d3[:, :, lo:hi],
                    mybir.ActivationFunctionType.Square,
                    accum_out=cacc[:, central_col : central_col + 1],
                )

            def emit_block(i, base, diffs):
                r, cs, ce = blocks[i]
                bw = ce - cs
                if cs == 0 and ce == W:
                    # full-width: 1-D row slice -> one contiguous r*W*4-byte
                    # descriptor per partition
                    src_p = pred.ap()[base : base + P * r].rearrange(
                        "(p r) w -> p (r w)", p=P
                    )
                    src_t = tgt.ap()[base : base + P * r].rearrange(
                        "(p r) w -> p (r w)", p=P
                    )
                else:
                    src_p = pred.ap()[base : base + P * r, cs:ce].rearrange(
                        "(p r) w -> p (r w)", p=P
                    )
                    src_t = tgt.ap()[base : base + P * r, cs:ce].rearrange(
                        "(p r) w -> p (r w)", p=P
                    )
                pt = io_pool.tile([P, rmax * W], F32, tag="pred")
                gt = io_pool.tile([P, rmax * W], F32, tag="tgt")
                nc.sync.dma_start(pt[:, : r * bw], src_p)
                nc.scalar.dma_start(gt[:, : r * bw], src_t)

                # Lagged Square for an earlier block goes AFTER this block's
                # target dma_start in the ACT queue.
                if i >= lag:
                    emit_central(i - lag, diffs)

                diff = work_pool.tile([P, rmax * W], F32, tag="diff")
                diffs[i] = diff
                nc.vector.tensor_sub(
                    diff[:, : r * bw], pt[:, : r * bw], gt[:, : r * bw]
                )
                d3 = diff[:, : r * bw].rearrange("p (r w) -> p r w", w=bw)

                # periphery abs-sums (DVE), one per band present in the block
                pcols = list(pieces[i][1])
                if cs < CS and min(ce, CS) > cs:
                    col = pcols.pop(0)
                    nc.vector.tensor_reduce(
                        pacc[:, col : col + 1],
                        d3[:, :, 0 : min(ce, CS) - cs],
                        axis=mybir.AxisListType.XY,
                        op=mybir.AluOpType.add,
                        apply_absolute_value=True,
                    )
                if ce > CE and max(cs, CE) < ce:
                    col = pcols.pop(0)
                    lo = max(cs, CE) - cs
                    nc.vector.tensor_reduce(
                        pacc[:, col : col + 1],
                        d3[:, :, lo : ce - cs],
                        axis=mybir.AxisListType.XY,
                        op=mybir.AluOpType.add,
                        apply_absolute_value=True,
                    )

            def body():
                diffs = {}
                base = 0
                for i, (r, cs, ce) in enumerate(blocks):
                    emit_block(i, base, diffs)
                    if ce == W:
                        base += P * r
                for i in range(max(NB - lag, 0), NB):
                    emit_central(i, diffs)

            if loop_n > 1:
                with tc.For_i(0, loop_n, 1):
                    body()
            else:
                body()

            # Tiny result DMAs on separate rings -> receipts in parallel.
            # cacc's writers all finish while the last block is still
            # draining, so its flush overlaps the end of the stream.
            nc.sync.dma_start(cacc_out.ap(), cacc[:])
            nc.scalar.dma_start(pacc_out.ap(), pacc[:])

    nc.compile()
    return nc


_CACHED_NC = None


def _get_program() -> bass.Bass:
    global _CACHED_NC
    if _CACHED_NC is None:
        _CACHED_NC = build_program()
    return _CACHED_NC


def shard_inputs(pred: np.ndarray, target: np.ndarray) -> list[dict]:
    in_maps = []
    for i in range(N_CORES):
        sl = slice(i * B_SHARD, (i + 1) * B_SHARD)
        in_maps.append(
            {
                "pred": np.ascontiguousarray(pred[sl]).reshape(ROWS, W),
                "target": np.ascontiguousarray(target[sl]).reshape(ROWS, W),
            }
        )
    return in_maps


def reduce_partials(results: list[dict]) -> np.ndarray:
    tot_sq = 0.0
    tot_abs = 0.0
    for r in results:
        tot_sq += r["cacc"].astype(np.float64).sum()
        tot_abs += r["pacc"].astype(np.float64).sum()
    loss = tot_sq / (B * H * CW) + tot_abs / (B * H * PW)
    return np.asarray(loss, dtype=np.float32)


def kernel(pred: np.ndarray, target: np.ndarray) -> np.ndarray:
    pred = np.asarray(pred, dtype=np.float32)
    target = np.asarray(target, dtype=np.float32)
    nc = _get_program()
    in_maps = shard_inputs(pred, target)
    res = run_bass_kernel_spmd(nc, in_maps, list(range(N_CORES)))
    return reduce_partials(res.results)
